# revision 2
# baseline (speedup 1.0000x reference)
"""Trainium2 Bass kernel for the DigitConvolutionalModel problem.

Math: out = relu(conv3x3(x) @ fc1_w.T + fc1_b) @ fc2_w.T + fc2_b
The 3x3 valid conv followed by a dense layer composes into a single
linear map, so conv_w and fc1_w are folded on the host into one
W1eff [128, 784] matrix. The device then runs two matmuls + bias/relu.

Sharding: pure data parallelism — batch split across 8 cores.
Each core's x shard is staged transposed ([784, 8192]) so the
contraction dim lands on SBUF partitions with contiguous DMA.
"""

import numpy as np

import concourse.bacc as bacc
import concourse.mybir as mybir
import concourse.tile as tile
from concourse.bass_utils import run_bass_kernel_spmd

N_CORES = 8
B = 65536
B_LOCAL = B // N_CORES  # 8192
K = 784                 # input features (28*28)
M1 = 128                # fc1 out
M2 = 10                 # fc2 out
KCS = [128] * 6 + [16]  # contraction chunks of 784

F32 = mybir.dt.float32
BF16 = mybir.dt.bfloat16

# matmul operand mode: "f32" (exact, 4 cyc/row) or "bf16" (cast during
# DMA, 1 cyc/row)
MODE = "bf16"
BT = 2048               # batch tile per DMA
NS = 512                # matmul moving-dim subtile (one PSUM bank)

_cache = {}


def _build_nc(mode=MODE, bt=BT):
    nc = bacc.Bacc("TRN2", target_bir_lowering=False, debug=False,
                   num_devices=N_CORES)
    mm_dt = BF16 if mode == "bf16" else F32
    x_d = nc.dram_tensor("x_t", [K, B_LOCAL], F32, kind="ExternalInput")
    w1_d = nc.dram_tensor("w1t", [K, M1], F32, kind="ExternalInput")
    b1_d = nc.dram_tensor("b1", [M1, 1], F32, kind="ExternalInput")
    w2_d = nc.dram_tensor("w2t", [M1, M2], F32, kind="ExternalInput")
    b2_d = nc.dram_tensor("b2", [M2, 1], F32, kind="ExternalInput")
    z_d = nc.dram_tensor("z_t", [M2, B_LOCAL], F32, kind="ExternalOutput")

    with tile.TileContext(nc) as tc:
        with (
            tc.tile_pool(name="static", bufs=1) as sp,
            tc.tile_pool(name="xp", bufs=2) as xp,
            tc.tile_pool(name="hp", bufs=3) as hp,
            tc.tile_pool(name="zp", bufs=3) as zp,
            tc.tile_pool(name="pp1", bufs=2, space="PSUM") as pp1,
            tc.tile_pool(name="pp2", bufs=2, space="PSUM") as pp2,
        ):
            w1s = []
            off = 0
            for kc, ks in enumerate(KCS):
                wt = sp.tile([ks, M1], mm_dt, tag=f"w1_{kc}")
                nc.gpsimd.dma_start(wt[:], w1_d[off:off + ks, :])
                w1s.append(wt)
                off += ks
            w2t = sp.tile([M1, M2], mm_dt, tag="w2")
            nc.gpsimd.dma_start(w2t[:], w2_d[:])
            b1t = sp.tile([M1, 1], F32, tag="b1")
            nc.gpsimd.dma_start(b1t[:], b1_d[:])
            b2t = sp.tile([M2, 1], F32, tag="b2")
            nc.gpsimd.dma_start(b2t[:], b2_d[:])

            for bt_i in range(B_LOCAL // bt):
                xts = []
                off = 0
                for kc, ks in enumerate(KCS):
                    xt = xp.tile([ks, bt], mm_dt, tag=f"x{kc}")
                    nc.gpsimd.dma_start(
                        xt[:], x_d[off:off + ks, bt_i * bt:(bt_i + 1) * bt])
                    xts.append(xt)
                    off += ks
                for ns in range(bt // NS):
                    sl = slice(ns * NS, (ns + 1) * NS)
                    ps1 = pp1.tile([M1, NS], F32, tag="ps1")
                    for kc in range(len(KCS)):
                        nc.tensor.matmul(
                            ps1[:],
                            w1s[kc][:],
                            xts[kc][:, sl],
                            start=(kc == 0),
                            stop=(kc == len(KCS) - 1),
                        )
                    h = hp.tile([M1, NS], mm_dt, tag="h")
                    nc.scalar.activation(
                        h[:], ps1[:], mybir.ActivationFunctionType.Relu,
                        bias=b1t[:])
                    ps2 = pp2.tile([M2, NS], F32, tag="ps2")
                    nc.tensor.matmul(
                        ps2[:], w2t[:], h[:], start=True, stop=True)
                    zt = zp.tile([M2, NS], F32, tag="z")
                    nc.vector.tensor_scalar_add(zt[:], ps2[:], b2t[:])
                    nc.gpsimd.dma_start(
                        z_d[:, bt_i * bt + ns * NS: bt_i * bt + (ns + 1) * NS],
                        zt[:])
    nc.compile()
    return nc


def _fold_weights(conv_w, fc1_w):
    """Fold 3x3 valid cross-correlation + fc1 into one [128, 784] matrix."""
    cw = np.asarray(conv_w, np.float64)
    f1 = np.asarray(fc1_w, np.float64).reshape(M1, 26, 26)
    W = np.zeros((M1, 28, 28), np.float64)
    for di in range(3):
        for dj in range(3):
            W[:, di:di + 26, dj:dj + 26] += cw[di, dj] * f1
    return W.reshape(M1, K).astype(np.float32)


def kernel(x, conv_w, fc1_w, fc1_b, fc2_w, fc2_b):
    if "nc" not in _cache:
        _cache["nc"] = _build_nc()
    nc = _cache["nc"]

    w1t = np.ascontiguousarray(_fold_weights(conv_w, fc1_w).T)  # [784, 128]
    b1 = np.ascontiguousarray(np.asarray(fc1_b, np.float32).reshape(M1, 1))
    w2t = np.ascontiguousarray(np.asarray(fc2_w, np.float32).T)  # [128, 10]
    b2 = np.ascontiguousarray(np.asarray(fc2_b, np.float32).reshape(M2, 1))
    x = np.asarray(x, np.float32)

    in_maps = []
    for c in range(N_CORES):
        xs = x[c * B_LOCAL:(c + 1) * B_LOCAL]
        in_maps.append({
            "x_t": np.ascontiguousarray(xs.T),
            "w1t": w1t, "b1": b1, "w2t": w2t, "b2": b2,
        })
    res = run_bass_kernel_spmd(nc, in_maps, list(range(N_CORES)))
    outs = [res.results[c]["z_t"].T for c in range(N_CORES)]
    return np.ascontiguousarray(np.concatenate(outs, axis=0), dtype=np.float32)
